# revision 9
# baseline (speedup 1.0000x reference)
"""NSA-style compressed + top-k block-sparse attention (MiniCPMSparseFlashAttention2)
for Trainium2, distributed over 8 NeuronCores.

Key reduction (validated against the reference): with KERNEL=32, STRIDE=16,
BLOCK=64, TOPK=4, INIT_BLOCKS=1, LOCAL_BLOCKS=2, the reference top-k selection
collapses to: query block qb attends to key blocks
    qb=0 -> {} (output exactly 0), qb=1 -> {0}, qb=2 -> {0,1},
    qb>=3 -> {0, qb-1, b*} where b* = argmax over b in [1, qb-2] of the
    max-pooled compressed-attention (stage 1) score.

This revision packs TWO tokens per dynamic matmul: a host-built DRAM table
holds, for every ordered pair of key blocks (b_e, b_o), a [128, 128] bf16
K-slab (keys of both blocks as PE stationary columns; contraction = d) and a
[128, 128] V-slab (pair keys on partitions; columns = d). One SWDGE
transpose-gather per 128-token wave fetches 64 K-pair + 64 V-pair slabs with
a single 128-entry index vector (V rows offset by 1024 in the combined
table). Stage-2 runs transposed in 4-chunk groups (512 rows sharing one
query block): one 512-wide matmul for the fixed keys (block 0 | qb-1), 32
pair matmuls for the dynamic logits, ones-matmul denominators, and the PV
accumulation mirrored the same way. Cross-token garbage quadrants in the
pair logits are killed with a +NEG column mask before the exp. Softmax
denominators are DMA'd out and the division happens on the host during the
unshard (masked rows clamp to zero).

Stage-1 stays numerically fp32-faithful without fp32 matmuls: logits use a
3-term bf16 hi/lo split (q_hi*k_hi + q_hi*k_lo + q_lo*k_hi, fp32 PSUM), and
the per-chunk head-sum of probs uses an exact bf16 hi/lo prob split with 0/1
selector stationaries (one column-32 PE tile per chunk pair).

Sharding: 8 cores = 2 KV heads x 4 query-block interleaves (core part p owns
query blocks p, p+4, ..., p+28 - balanced). One uniform program for all cores.
"""
import sys
sys.path.insert(0, '/opt/trn_rl_repo')
import math
import re as _re
import numpy as np
import ml_dtypes

import concourse.bass as bass
import concourse.tile as tile
import concourse.mybir as mybir
from concourse.bass_utils import run_bass_kernel_spmd
from concourse.library_config import mlp
from concourse.library_overlay import lower_extended_insts

dt = mybir.dt
F32 = dt.float32
BF16 = dt.bfloat16
I16 = dt.int16
AF = mybir.ActivationFunctionType
ALU = mybir.AluOpType
AX = mybir.AxisListType

S, HQ, HKV, D = 2048, 16, 2, 128
G = HQ // HKV                     # 8 query heads per kv head
KERNEL, STRIDE, BLOCK = 32, 16, 64
B = S // BLOCK                    # 32 kv blocks
C = (S - KERNEL) // STRIDE + 1    # 127 compressed keys
NCORES = 8
NPART = 4                         # seq-parallel parts per kv head
NQ = S // NPART                   # 512 queries per core
NCH = NQ * G // 128               # 32 chunks of 128 (query,g) rows
NST = NQ // 128                   # 4 waves of 128 tokens
QPC = 16                          # queries per chunk
WCH = NCH // NST                  # 8 chunks per wave
NGRP = NCH // 4                   # 8 groups of 4 chunks (shared query block)
GC = 512                          # columns (rows of attention) per group
SCALE = 1.0 / math.sqrt(D)
SCALE1 = SCALE / KERNEL           # stage-1 kcmp left unnormalized (sum not mean)
NEG = -1e30


# ---------------------------------------------------------------------------
# Workaround for this container's bass/walrus build: TileContext's exit drain
# carries all end-of-kernel sem waits on one CTRL Drain instruction, which
# this walrus rejects ("Too many sync wait commands"). Emit the waits as
# separate SP wait_ge instructions and a bare drain instead.
def _patched_drain_and_barrier(self, tick_clock, wait_clock):
    nc = self.nc
    ticks = [int(v) for v in _re.findall(r"-?\d+", repr(tick_clock.global_clock))]
    sems = self.sems.allocated()
    for proc, sem in sems.items():
        t = ticks[proc]
        if t > 0:
            nc.sync.wait_ge(sem, t * (16 if "DMA" in sem.name else 1))
    nc.sync.drain()
    nc.all_engine_barrier()
    popped = nc._tile_sem_poison_stack.pop()
    assert popped is self._sem_poison
    nc.clear_and_free_semaphores(list(sems.values()))
    nc.all_engine_barrier()


tile.TileContext._drain_and_barrier = _patched_drain_and_barrier


def _split_excess_waits(nc, keep=1):
    """Walrus here rejects >1 sync wait on several instruction encodings.
    Move excess waits onto injected same-engine InstEventSemaphore
    instructions placed immediately before."""
    for f in nc.m.functions:
        for bb in f.blocks:
            old = list(bb.instructions)
            if not any(i.sync_info and i.sync_info.on_wait and
                       len(i.sync_info.on_wait) > keep for i in old):
                continue
            new = []
            for inst in old:
                si = inst.sync_info
                if si and si.on_wait and len(si.on_wait) > keep:
                    waits = list(si.on_wait)
                    excess, kept = waits[:-keep], waits[-keep:]
                    for w in excess:
                        new.append(mybir.InstEventSemaphore(
                            name=nc.get_next_instruction_name(),
                            engine=inst.engine, ins=[], outs=[],
                            sync_info=mybir.SyncInfo(on_wait=[w], on_update=[]),
                        ))
                    inst.sync_info = mybir.SyncInfo(
                        on_wait=kept, on_update=list(si.on_update))
                new.append(inst)
            bb.instructions = new


def build_program(_for_sim=False):
    nc = bass.Bass("TRN2", num_devices=NCORES,
                   dynamic_dma_scratch_size=32768)
    tensors = dict(
        qTb=nc.dram_tensor("qTb", [128, NQ * G], BF16, kind="ExternalInput"),
        qTl=nc.dram_tensor("qTl", [128, NQ * G], BF16, kind="ExternalInput"),
        kT=nc.dram_tensor("kT", [128, S], F32, kind="ExternalInput"),
        kabT=nc.dram_tensor("kabT", [128, NGRP * 128], BF16, kind="ExternalInput"),
        vab=nc.dram_tensor("vab", [128, NGRP * 128], BF16, kind="ExternalInput"),
        b2a=nc.dram_tensor("b2a", [128, NGRP], F32, kind="ExternalInput"),
        b2b=nc.dram_tensor("b2b", [128, NGRP], F32, kind="ExternalInput"),
        bias1=nc.dram_tensor("bias1", [128, NCH * C], BF16, kind="ExternalInput"),
        identB=nc.dram_tensor("identB", [128, 128], BF16, kind="ExternalInput"),
        mmid=nc.dram_tensor("mmid", [128, NST * B], F32, kind="ExternalInput"),
        iotab=nc.dram_tensor("iotab", [128, B], F32, kind="ExternalInput"),
        sel32e=nc.dram_tensor("sel32e", [128, 32], BF16, kind="ExternalInput"),
        sel32o=nc.dram_tensor("sel32o", [128, 32], BF16, kind="ExternalInput"),
        selKE=nc.dram_tensor("selKE", [128, 128], BF16, kind="ExternalInput"),
        qmE=nc.dram_tensor("qmE", [128, 8], BF16, kind="ExternalInput"),
        selP2=nc.dram_tensor("selP2", [128, 128], BF16, kind="ExternalInput"),
        qmask8=nc.dram_tensor("qmask8", [128, 8], BF16, kind="ExternalInput"),
        off1=nc.dram_tensor("off1", [128, 8], F32, kind="ExternalInput"),
        maskc=nc.dram_tensor("maskc", [128, GC], F32, kind="ExternalInput"),
        onesb=nc.dram_tensor("onesb", [128, 1], BF16, kind="ExternalInput"),
        eps20=nc.dram_tensor("eps20", [128, 1], F32, kind="ExternalInput"),
        k16=nc.dram_tensor("k16", [B, 64 * 128], BF16, kind="ExternalInput"),
        vptab=nc.dram_tensor("vptab", [2 * B * B, 64 * 128], BF16,
                             kind="ExternalInput"),
        out=nc.dram_tensor("out", [128, NCH * 128], BF16, kind="ExternalOutput"),
        outl=nc.dram_tensor("outl", [1, NCH * 128], F32, kind="ExternalOutput"),
    )
    with tile.TileContext(nc) as tc:
        _build_body(nc, tc, tensors)
    if not _for_sim:
        lower_extended_insts(nc)
        _split_excess_waits(nc)
    return nc


def _build_body(nc, tc, t):
    from contextlib import ExitStack
    with ExitStack() as ctx:
        const = ctx.enter_context(tc.tile_pool(name="const", bufs=1))

        nc.gpsimd.load_library(mlp)

        kT = const.tile([128, S // 16, 16], F32)
        qTb = const.tile([128, NQ * G], BF16)
        qTl = const.tile([128, NQ * G], BF16)
        kabT = const.tile([128, NGRP * 128], BF16)
        vab = const.tile([128, NGRP * 128], BF16)
        b2a = const.tile([128, NGRP], F32)
        b2b = const.tile([128, NGRP], F32)
        bias1 = const.tile([128, NCH * C], BF16)
        identB = const.tile([128, 128], BF16)
        mmid = const.tile([128, NST * B], F32)
        iotab = const.tile([128, B], F32)
        sel32e = const.tile([128, 32], BF16)
        sel32o = const.tile([128, 32], BF16)
        selKE = const.tile([128, 128], BF16)
        qmE = const.tile([128, 8], BF16)
        selP2 = const.tile([128, 128], BF16)
        qmask8 = const.tile([128, 8], BF16)
        off1 = const.tile([128, 8], F32)
        maskc = const.tile([128, GC], F32)
        onesb = const.tile([128, 1], BF16)
        eps20 = const.tile([128, 1], F32)
        # kT first: the stage-1 front needs it before anything else
        nc.sync.dma_start(kT[:], t["kT"][:])
        for j in range(4):
            nc.sync.dma_start(qTb[:, j * 1024:(j + 1) * 1024],
                              t["qTb"][:, j * 1024:(j + 1) * 1024])
            nc.sync.dma_start(qTl[:, j * 1024:(j + 1) * 1024],
                              t["qTl"][:, j * 1024:(j + 1) * 1024])
            nc.sync.dma_start(bias1[:, j * 1016:(j + 1) * 1016],
                              t["bias1"][:, j * 1016:(j + 1) * 1016])
        nc.sync.dma_start(kabT[:], t["kabT"][:])
        nc.sync.dma_start(vab[:], t["vab"][:])
        for nm, tl in [("b2a", b2a), ("b2b", b2b), ("identB", identB),
                       ("mmid", mmid), ("iotab", iotab),
                       ("sel32e", sel32e), ("sel32o", sel32o),
                       ("selKE", selKE), ("qmE", qmE),
                       ("selP2", selP2), ("qmask8", qmask8), ("off1", off1),
                       ("maskc", maskc), ("onesb", onesb), ("eps20", eps20)]:
            nc.sync.dma_start(tl[:], t[nm][:])
        out_d = t["out"]
        outl_d = t["outl"]

        # ---- compressed keys: kcmpT[d, c] = sum_{j<32} kT[d, 16c+j] -------
        # computed in fp32, then split hi/lo into bf16 for the 3-term matmul
        half = const.tile([128, 128], F32)
        nc.vector.tensor_reduce(half[:], kT[:], axis=AX.X, op=ALU.add)
        kcmpT = const.tile([128, C], F32)
        nc.vector.tensor_add(kcmpT[:], half[:, 0:C], half[:, 1:C + 1])
        khB = const.tile([128, C], BF16)
        nc.vector.tensor_copy(khB[:], kcmpT[:])
        khF = const.tile([128, C], F32)
        nc.vector.tensor_copy(khF[:], khB[:])
        klB = const.tile([128, C], BF16)
        nc.vector.tensor_tensor(klB[:], kcmpT[:], khF[:], op=ALU.subtract)

        gidx = ctx.enter_context(tc.tile_pool(name="gidx", bufs=2))
        kgp = ctx.enter_context(tc.tile_pool(name="kgp", bufs=3))
        vgp = ctx.enter_context(tc.tile_pool(name="vgp", bufs=3))
        s1 = ctx.enter_context(tc.tile_pool(name="s1", bufs=6))
        s1b = ctx.enter_context(tc.tile_pool(name="s1b", bufs=4))
        s2 = ctx.enter_context(tc.tile_pool(name="s2", bufs=3))
        s2o = ctx.enter_context(tc.tile_pool(name="s2o", bufs=2))

        gathered = {}
        with tc.tile_pool(name="ps_lg1", bufs=2, space="PSUM") as ps_lg1, \
             tc.tile_pool(name="ps_sc", bufs=2, space="PSUM") as ps_sc, \
             tc.tile_pool(name="ps_bt", bufs=2, space="PSUM") as ps_bt:

            # ================= stage 1: scores + argmax block ==============
            score_ps = [None] * NST

            phpl = {}
            l12h = [None]

            def s1_front(ch):
                st, sub = divmod(ch, WCH)
                co = 128 * ch
                lg1 = ps_lg1.tile([128, C], F32, tag="lg1")
                nc.tensor.matmul(lg1[:], identB[:],
                                 bias1[:, C * ch:C * ch + C],
                                 start=True, stop=False)
                nc.tensor.matmul(lg1[:], qTb[:, co:co + 128], khB[:],
                                 start=False, stop=False)
                nc.tensor.matmul(lg1[:], qTb[:, co:co + 128], klB[:],
                                 start=False, stop=False)
                nc.tensor.matmul(lg1[:], qTl[:, co:co + 128], khB[:],
                                 start=False, stop=True)
                e1f = s1.tile([128, C], F32, tag="e1f")
                nc.scalar.activation(e1f[:], lg1[:], AF.Exp, scale=SCALE1)
                if ch % 2 == 0:
                    l12h[0] = s1b.tile([128, 2], F32, tag="l12", name="l12")
                l12 = l12h[0]
                nc.vector.tensor_reduce(l12[:, ch % 2:ch % 2 + 1], e1f[:],
                                        axis=AX.X, op=ALU.add)
                if ch % 2 == 1:
                    nln2 = s1b.tile([128, 2], F32, tag="nln2")
                    nc.scalar.activation(nln2[:], l12[:], AF.Ln,
                                         bias=eps20[:, 0:1])
                    nc.scalar.activation(nln2[:], nln2[:], AF.Copy, scale=-1.0)
                    l12h[1:] = [nln2]
                phpl[ch] = (lg1, None)

            def s1_probs(ch):
                lg1, _ = phpl[ch]
                nln2 = l12h[1]
                p1 = s1.tile([128, C], F32, tag="p1")
                nc.scalar.activation(p1[:], lg1[:], AF.Exp, scale=SCALE1,
                                     bias=nln2[:, ch % 2:ch % 2 + 1])
                ph = s1.tile([128, C], BF16, tag="ph")
                nc.vector.tensor_copy(ph[:], p1[:])
                pl = s1.tile([128, C], BF16, tag="pl")
                nc.vector.tensor_tensor(pl[:], p1[:], ph[:], op=ALU.subtract)
                phpl[ch] = (ph, pl)

            def s1_score(ch):
                st, sub = divmod(ch, WCH)
                ph, pl = phpl.pop(ch)
                if sub == 0:
                    score_ps[st] = ps_sc.tile([128, 128], F32, tag="score",
                                              name="score")
                j = sub // 2
                sel = sel32e if sub % 2 == 0 else sel32o
                nc.tensor.matmul(score_ps[st][32 * j:32 * j + 32, 0:C],
                                 sel[:], ph[:],
                                 start=(sub % 2 == 0), stop=False,
                                 tile_position=(0, 32 * j))
                nc.tensor.matmul(score_ps[st][32 * j:32 * j + 32, 0:C],
                                 sel[:], pl[:],
                                 start=False, stop=(sub % 2 == 1),
                                 tile_position=(0, 32 * j))
                if sub == WCH - 1:
                    _argmax_gather(st)

            def _argmax_gather(st):
                score = score_ps[st]
                blk = s1.tile([128, B], F32, tag="blk")
                nc.vector.tensor_copy(blk[:], score[:, 0:125:4])
                nc.vector.tensor_tensor(blk[:], blk[:], score[:, 1:126:4],
                                        op=ALU.max)
                nc.vector.tensor_tensor(blk[:], blk[:], score[:, 2:127:4],
                                        op=ALU.max)
                nc.vector.tensor_tensor(blk[:, 0:31], blk[:, 0:31],
                                        score[:, 3:127:4], op=ALU.max)
                nc.vector.tensor_tensor(blk[:, 1:32], blk[:, 1:32],
                                        score[:, 3:127:4], op=ALU.max)
                nc.vector.tensor_add(blk[:], blk[:], mmid[:, B * st:B * st + B])
                mx = s1b.tile([128, 1], F32, tag="mx")
                nc.vector.tensor_reduce(mx[:], blk[:], axis=AX.X, op=ALU.max)
                enc = s1.tile([128, B], F32, tag="enc")
                nc.vector.tensor_scalar(enc[:], blk[:], mx[:], 1024.0,
                                        op0=ALU.is_lt, op1=ALU.mult)
                nc.vector.tensor_tensor(enc[:], enc[:], iotab[:], op=ALU.add)
                bsf = s1b.tile([128, 1], F32, tag="bsf")
                nc.vector.tensor_reduce(bsf[:], enc[:], axis=AX.X, op=ALU.min)
                # K block idx: i=j -> b(even token 2j), i=64+j -> b(odd)
                rqK = s1b.tile([128, 8], BF16, tag="rqK")
                nc.vector.tensor_tensor(rqK[:], bsf[:].to_broadcast([128, 8]),
                                        qmE[:], op=ALU.mult)
                ktpK = ps_bt.tile([128, 8], F32, tag="ktpK")
                nc.tensor.matmul(ktpK[:], selKE[:], rqK[:],
                                 start=True, stop=True)
                idxK = gidx.tile([128, 8], I16, tag="idxK")
                nc.vector.tensor_copy(idxK[:], ktpK[:])
                # V half-slab idx: i=j -> 2*(32*b(2j)+b(2j+1)), i=64+j -> +1
                rqV = s1b.tile([128, 8], BF16, tag="rqV")
                nc.vector.tensor_tensor(rqV[:], bsf[:].to_broadcast([128, 8]),
                                        qmask8[:], op=ALU.mult)
                ktpV = ps_bt.tile([128, 8], F32, tag="ktpV")
                nc.tensor.matmul(ktpV[:], selP2[:], rqV[:],
                                 start=True, stop=True)
                t8 = s1b.tile([128, 8], F32, tag="t8")
                nc.vector.tensor_add(t8[:], ktpV[:], off1[:])
                idxV = gidx.tile([128, 8], I16, tag="idxV")
                nc.vector.tensor_copy(idxV[:], t8[:])
                kg = kgp.tile([128, 64, 128], BF16, tag="kg")
                nc.gpsimd.dma_gather(kg[:], t["k16"][:], idxK[:],
                                     128, 128, 64 * 128, transpose=True)
                vg = vgp.tile([128, 64, 128], BF16, tag="vg")
                nc.gpsimd.dma_gather(vg[:], t["vptab"][:], idxV[:],
                                     128, 128, 64 * 128, transpose=True)
                gathered[st] = (kg, vg)

            LAG = 4
            for ch in range(NCH + LAG):
                if ch < NCH:
                    s1_front(ch)
                    if ch % 2 == 1:
                        s1_probs(ch - 1)
                        s1_probs(ch)
                if ch >= LAG:
                    s1_score(ch - LAG)

        # ================= stage 2: block-sparse attention =================
        with tc.tile_pool(name="psA", bufs=2, space="PSUM") as psA, \
             tc.tile_pool(name="psB", bufs=2, space="PSUM") as psB, \
             tc.tile_pool(name="psO", bufs=2, space="PSUM") as psO, \
             tc.tile_pool(name="psL", bufs=2, space="PSUM") as psL:

            s2st = {}

            def s2_front(g):
                kg, vg = gathered[g // 2]
                jo = 32 * (g % 2)          # wave-pair offset of this group
                co = GC * g
                # fixed 128 keys (block0 | block qb-1), transposed logits
                lgA = psA.tile([128, GC], F32, tag="lgA")
                nc.tensor.matmul(lgA[:], kabT[:, 128 * g:128 * g + 128],
                                 qTb[:, co:co + GC], start=True, stop=True)
                # dynamic pair logits: 2 tokens x 64 keys per matmul
                lgB = psB.tile([128, GC], F32, tag="lgB")
                for j in range(32):
                    jj = jo + j
                    nc.tensor.matmul(
                        lgB[:, 16 * j:16 * j + 16],
                        kg[:, :, jj:jj + 65:64],
                        qTb[:, co + 16 * j:co + 16 * j + 16],
                        start=True, stop=True)
                eA = s2.tile([128, GC], BF16, tag="eA")
                nc.scalar.activation(eA[:], lgA[:], AF.Exp, scale=SCALE,
                                     bias=b2a[:, g:g + 1])
                # kill cross-token quadrants, then exp
                nc.vector.tensor_add(lgB[:], lgB[:], maskc[:])
                ptB = s2.tile([128, GC], BF16, tag="ptB")
                nc.scalar.activation(ptB[:], lgB[:], AF.Exp, scale=SCALE,
                                     bias=b2b[:, g:g + 1])
                s2st[g] = (eA, ptB)

            def s2_tail(g):
                kg, vg = gathered[g // 2]
                jo = 32 * (g % 2)
                co = GC * g
                eA, ptB = s2st.pop(g)
                # softmax denominators (column sums over both key groups)
                lr = psL.tile([1, GC], F32, tag="lr")
                nc.tensor.matmul(lr[:], onesb[:], eA[:], start=True, stop=False)
                nc.tensor.matmul(lr[:], onesb[:], ptB[:], start=False, stop=True)
                louts = s2o.tile([1, GC], F32, tag="louts")
                nc.vector.tensor_copy(louts[:], lr[:])
                nc.sync.dma_start(outl_d[:, co:co + GC], louts[:])
                # PV accumulation (transposed output [d, row])
                oT = psO.tile([128, GC], F32, tag="oT")
                nc.tensor.matmul(oT[:], vab[:, 128 * g:128 * g + 128], eA[:],
                                 start=True, stop=False)
                for j in range(32):
                    jj = jo + j
                    nc.tensor.matmul(
                        oT[:, 16 * j:16 * j + 16],
                        vg[:, :, jj:jj + 65:64],
                        ptB[:, 16 * j:16 * j + 16],
                        start=False, stop=(j == 31))
                outc = s2o.tile([128, GC], BF16, tag="outc")
                nc.vector.tensor_copy(outc[:], oT[:])
                nc.sync.dma_start(out_d[:, co:co + GC], outc[:])

            for g in range(NGRP + 1):
                if g < NGRP:
                    s2_front(g)
                if g >= 1:
                    s2_tail(g - 1)


_NC_CACHE = None


def _get_program():
    global _NC_CACHE
    if _NC_CACHE is None:
        _NC_CACHE = build_program()
    return _NC_CACHE


def _make_tables(k, v, h):
    """Per-KV-head gather tables (16KB elems):
    k16   [B, 64*128]      K block natural [key, d] -> transpose-gather [d,key]
    vptab [2*B*B, 64*128]  V-pair half-slabs, row 2p+h (p = 32*be+bo):
                           elem[e, pp] = V_{b(pp%2)}[pp//2, d=2e+h]
                           (pp = interleaved pair keys 2k+t, d split by parity)
    """
    kh = k[:, h, :].astype(ml_dtypes.bfloat16)
    vh = v[:, h, :].astype(ml_dtypes.bfloat16)
    k16 = np.ascontiguousarray(kh.reshape(B, BLOCK * D))
    Vt2 = vh.reshape(B, BLOCK, D).transpose(0, 2, 1)       # [b, d, key]
    A = np.broadcast_to(Vt2[:, None, :, :], (B, B, D, BLOCK))
    Bb = np.broadcast_to(Vt2[None, :, :, :], (B, B, D, BLOCK))
    WT = np.stack([A, Bb], axis=-1).reshape(B * B, D, 128)  # [p, d, pp]
    vptab = np.ascontiguousarray(
        WT.reshape(B * B, 64, 2, 128).transpose(0, 2, 1, 3)
    ).reshape(2 * B * B, 64 * 128)
    return k16, vptab


def _make_core_inputs(q, k, v, h, part, k16, vptab):
    qbs = [part + NPART * j for j in range(NQ // BLOCK)]
    ls = np.concatenate([np.arange(BLOCK * b, BLOCK * b + BLOCK) for b in qbs])
    qc = q[ls][:, h * G:(h + 1) * G, :].reshape(NQ * G, D)
    qT = np.ascontiguousarray(qc.T)
    qTb = qT.astype(ml_dtypes.bfloat16)
    qTl = (qT - qTb.astype(np.float32)).astype(ml_dtypes.bfloat16)
    kh = k[:, h, :]
    kT = np.ascontiguousarray(kh.T)
    vh = v[:, h, :]

    qb_g = np.array(qbs)                       # query block per group
    qbf = np.maximum(qb_g - 1, 0)

    kabT = np.empty((128, NGRP * 128), np.float32)
    vab = np.empty((128, NGRP * 128), np.float32)
    for g in range(NGRP):
        kabT[:, 128 * g:128 * g + 64] = kT[:, 0:64]
        kabT[:, 128 * g + 64:128 * g + 128] = \
            kT[:, 64 * qbf[g]:64 * qbf[g] + 64]
        vab[0:64, 128 * g:128 * g + 128] = vh[0:64]
        vab[64:128, 128 * g:128 * g + 128] = \
            vh[64 * qbf[g]:64 * qbf[g] + 64]
    kabT = kabT.astype(ml_dtypes.bfloat16)
    vab = vab.astype(ml_dtypes.bfloat16)

    b2a = np.empty((128, NGRP), np.float32)
    b2a[0:64] = np.where(qb_g >= 1, 0.0, NEG)[None, :]
    b2a[64:128] = np.where(qb_g >= 2, 0.0, NEG)[None, :]
    b2b = np.broadcast_to(
        np.where(qb_g >= 3, 0.0, NEG).astype(np.float32), (128, NGRP)).copy()

    # stage-1 visibility bias: compressed key c visible iff 16c+31 <= s
    rows_s = ls[(QPC * np.arange(NCH)[None, :] + np.arange(128)[:, None] // G)]
    thr = np.floor((rows_s.astype(np.float64) - (KERNEL - 1)) / STRIDE)
    vis = np.arange(C)[None, :, None] <= thr.T[:, None, :]  # [NCH, C, 128]
    bias1 = np.where(vis, 0.0, NEG)
    bias1 = np.ascontiguousarray(
        bias1.transpose(2, 0, 1).reshape(128, NCH * C)).astype(
            ml_dtypes.bfloat16)
    identB = np.eye(128, dtype=ml_dtypes.bfloat16)

    qb_of_li = ls // BLOCK
    mmid = np.full((128, NST * B), -1e38, np.float32)
    for sti in range(NST):
        qb_rows = qb_of_li[128 * sti + np.arange(128)]
        allowed = (np.arange(B)[None, :] >= 1) & \
                  (np.arange(B)[None, :] <= qb_rows[:, None] - 2)
        allowed[~allowed.any(axis=1), 1] = True
        mmid[:, B * sti:B * sti + B] = np.where(allowed, 0.0, -1e38)

    iotab = np.broadcast_to(np.arange(B, dtype=np.float32), (128, B)).copy()
    r128 = np.arange(128)
    sel32e = (r128[:, None] // 8 == np.arange(32)[None, :]
              ).astype(ml_dtypes.bfloat16)
    sel32o = (r128[:, None] // 8 == np.arange(32)[None, :] - 16
              ).astype(ml_dtypes.bfloat16)
    selKE = ((r128[:, None] % 32) // 2 == r128[None, :] % 16
             ).astype(ml_dtypes.bfloat16)
    qmE = ((r128[:, None] // 32 == np.arange(8)[None, :] % 4) &
           (r128[:, None] % 2 == np.arange(8)[None, :] // 4)
           ).astype(ml_dtypes.bfloat16)
    selP2 = (64.0 * (r128[:, None] % 32 == 2 * (r128[None, :] % 16)) +
             2.0 * (r128[:, None] % 32 == 2 * (r128[None, :] % 16) + 1)
             ).astype(ml_dtypes.bfloat16)
    qmask8 = (r128[:, None] // 32 == np.arange(8)[None, :] % 4
              ).astype(ml_dtypes.bfloat16)
    off1 = np.broadcast_to(
        1.0 * (np.arange(8) >= 4).astype(np.float32), (128, 8)).copy()
    cc = np.arange(GC)
    maskc = np.where(r128[:, None] % 2 == (cc[None, :] % 16) // 8,
                     0.0, NEG).astype(np.float32)
    onesb = np.ones((128, 1), ml_dtypes.bfloat16)

    return {"qTb": qTb, "qTl": qTl, "kT": kT, "kabT": kabT, "vab": vab,
            "b2a": b2a, "b2b": b2b, "bias1": bias1, "identB": identB,
            "mmid": mmid, "iotab": iotab, "sel32e": sel32e, "sel32o": sel32o,
            "selKE": selKE, "qmE": qmE, "selP2": selP2,
            "qmask8": qmask8, "off1": off1,
            "maskc": maskc, "onesb": onesb,
            "eps20": np.full((128, 1), 1e-20, np.float32),
            "k16": k16, "vptab": vptab}, ls


def kernel(q, k, v, _profile=False):
    q = np.asarray(q, dtype=np.float32)
    k = np.asarray(k, dtype=np.float32)
    v = np.asarray(v, dtype=np.float32)
    nc = _get_program()

    tabs = [_make_tables(k, v, h) for h in range(HKV)]

    in_maps = []
    ls_per_core = []
    for c in range(NCORES):
        h, part = divmod(c, NPART)
        im, ls = _make_core_inputs(q, k, v, h, part, *tabs[h])
        in_maps.append(im)
        ls_per_core.append(ls)

    kw = dict(trace=True) if _profile else {}
    res = run_bass_kernel_spmd(nc, in_maps, list(range(NCORES)), **kw)

    out = np.zeros((S, HQ, D), dtype=np.float32)
    for c in range(NCORES):
        h, part = divmod(c, NPART)
        oc = np.asarray(res.results[c]["out"], dtype=np.float32)  # [128, 4096]
        l = np.asarray(res.results[c]["outl"], dtype=np.float32)  # [1, 4096]
        oc = oc / np.maximum(l, 1e-30)
        ocr = oc.reshape(D, NCH, QPC, G).transpose(1, 2, 3, 0)  # [NCH,16,G,D]
        out[ls_per_core[c], h * G:(h + 1) * G, :] = ocr.reshape(NQ, G, D)
    if _profile:
        return out, res
    return out


# revision 10
# speedup vs baseline: 1.0931x; 1.0931x over previous
"""NSA-style compressed + top-k block-sparse attention (MiniCPMSparseFlashAttention2)
for Trainium2, distributed over 8 NeuronCores.

Key reduction (validated against the reference): with KERNEL=32, STRIDE=16,
BLOCK=64, TOPK=4, INIT_BLOCKS=1, LOCAL_BLOCKS=2, the reference top-k selection
collapses to: query block qb attends to key blocks
    qb=0 -> {} (output exactly 0), qb=1 -> {0}, qb=2 -> {0,1},
    qb>=3 -> {0, qb-1, b*} where b* = argmax over b in [1, qb-2] of the
    max-pooled compressed-attention (stage 1) score.

This revision packs TWO tokens per dynamic matmul: a host-built DRAM table
holds, for every ordered pair of key blocks (b_e, b_o), a [128, 128] bf16
K-slab (keys of both blocks as PE stationary columns; contraction = d) and a
[128, 128] V-slab (pair keys on partitions; columns = d). One SWDGE
transpose-gather per 128-token wave fetches 64 K-pair + 64 V-pair slabs with
a single 128-entry index vector (V rows offset by 1024 in the combined
table). Stage-2 runs transposed in 4-chunk groups (512 rows sharing one
query block): one 512-wide matmul for the fixed keys (block 0 | qb-1), 32
pair matmuls for the dynamic logits, ones-matmul denominators, and the PV
accumulation mirrored the same way. Cross-token garbage quadrants in the
pair logits are killed with a +NEG column mask before the exp. Softmax
denominators are DMA'd out and the division happens on the host during the
unshard (masked rows clamp to zero).

Stage-1 stays numerically fp32-faithful without fp32 matmuls: logits use a
3-term bf16 hi/lo split (q_hi*k_hi + q_hi*k_lo + q_lo*k_hi, fp32 PSUM), and
the per-chunk head-sum of probs uses an exact bf16 hi/lo prob split with 0/1
selector stationaries (one column-32 PE tile per chunk pair).

Sharding: 8 cores = 2 KV heads x 4 query-block interleaves (core part p owns
query blocks p, p+4, ..., p+28 - balanced). One uniform program for all cores.
"""
import sys
sys.path.insert(0, '/opt/trn_rl_repo')
import math
import re as _re
import numpy as np
import ml_dtypes

import concourse.bass as bass
import concourse.tile as tile
import concourse.mybir as mybir
from concourse.bass_utils import run_bass_kernel_spmd
from concourse.library_config import mlp
from concourse.library_overlay import lower_extended_insts

dt = mybir.dt
F32 = dt.float32
BF16 = dt.bfloat16
I16 = dt.int16
AF = mybir.ActivationFunctionType
ALU = mybir.AluOpType
AX = mybir.AxisListType

S, HQ, HKV, D = 2048, 16, 2, 128
G = HQ // HKV                     # 8 query heads per kv head
KERNEL, STRIDE, BLOCK = 32, 16, 64
B = S // BLOCK                    # 32 kv blocks
C = (S - KERNEL) // STRIDE + 1    # 127 compressed keys
NCORES = 8
NPART = 4                         # seq-parallel parts per kv head
NQ = S // NPART                   # 512 queries per core
NCH = NQ * G // 128               # 32 chunks of 128 (query,g) rows
NST = NQ // 128                   # 4 waves of 128 tokens
QPC = 16                          # queries per chunk
WCH = NCH // NST                  # 8 chunks per wave
NGRP = NCH // 4                   # 8 groups of 4 chunks (shared query block)
GC = 512                          # columns (rows of attention) per group
QW = 1024 + 1024 + 8 * C          # per-wave blob: qTb | qTl | bias1
B16 = dict(kabT=0, vab=1024, sel32e=2048, sel32o=2080, selKE=2112,
           qmE=2240, selP2=2248, qmask8=2376, onesb=2384, identB=2385)
W16 = 2513
F32O = dict(b2a=0, b2b=NGRP, mmid=2 * NGRP, iotab=2 * NGRP + NST * B,
            off1=2 * NGRP + NST * B + B, maskc=2 * NGRP + NST * B + B + 8,
            eps20=2 * NGRP + NST * B + B + 8 + GC)
W32 = 2 * NGRP + NST * B + B + 8 + GC + 1
SCALE = 1.0 / math.sqrt(D)
SCALE1 = SCALE / KERNEL           # stage-1 kcmp left unnormalized (sum not mean)
NEG = -1e30


# ---------------------------------------------------------------------------
# Workaround for this container's bass/walrus build: TileContext's exit drain
# carries all end-of-kernel sem waits on one CTRL Drain instruction, which
# this walrus rejects ("Too many sync wait commands"). Emit the waits as
# separate SP wait_ge instructions and a bare drain instead.
def _patched_drain_and_barrier(self, tick_clock, wait_clock):
    nc = self.nc
    ticks = [int(v) for v in _re.findall(r"-?\d+", repr(tick_clock.global_clock))]
    sems = self.sems.allocated()
    for proc, sem in sems.items():
        t = ticks[proc]
        if t > 0:
            nc.sync.wait_ge(sem, t * (16 if "DMA" in sem.name else 1))
    nc.sync.drain()
    nc.all_engine_barrier()
    popped = nc._tile_sem_poison_stack.pop()
    assert popped is self._sem_poison
    nc.clear_and_free_semaphores(list(sems.values()))
    nc.all_engine_barrier()


tile.TileContext._drain_and_barrier = _patched_drain_and_barrier


def _split_excess_waits(nc, keep=1):
    """Walrus here rejects >1 sync wait on several instruction encodings.
    Move excess waits onto injected same-engine InstEventSemaphore
    instructions placed immediately before."""
    for f in nc.m.functions:
        for bb in f.blocks:
            old = list(bb.instructions)
            if not any(i.sync_info and i.sync_info.on_wait and
                       len(i.sync_info.on_wait) > keep for i in old):
                continue
            new = []
            for inst in old:
                si = inst.sync_info
                if si and si.on_wait and len(si.on_wait) > keep:
                    waits = list(si.on_wait)
                    excess, kept = waits[:-keep], waits[-keep:]
                    for w in excess:
                        new.append(mybir.InstEventSemaphore(
                            name=nc.get_next_instruction_name(),
                            engine=inst.engine, ins=[], outs=[],
                            sync_info=mybir.SyncInfo(on_wait=[w], on_update=[]),
                        ))
                    inst.sync_info = mybir.SyncInfo(
                        on_wait=kept, on_update=list(si.on_update))
                new.append(inst)
            bb.instructions = new


def build_program(_for_sim=False):
    nc = bass.Bass("TRN2", num_devices=NCORES,
                   dynamic_dma_scratch_size=32768)
    tensors = dict(
        kT=nc.dram_tensor("kT", [128, S], F32, kind="ExternalInput"),
        qcb=nc.dram_tensor("qcb", [128, NST * QW], BF16, kind="ExternalInput"),
        cb16=nc.dram_tensor("cb16", [128, W16], BF16, kind="ExternalInput"),
        cf32=nc.dram_tensor("cf32", [128, W32], F32, kind="ExternalInput"),
        k16=nc.dram_tensor("k16", [B, 64 * 128], BF16, kind="ExternalInput"),
        vptab=nc.dram_tensor("vptab", [2 * B * B, 64 * 128], BF16,
                             kind="ExternalInput"),
        out=nc.dram_tensor("out", [128, NCH * 128], BF16, kind="ExternalOutput"),
        outl=nc.dram_tensor("outl", [1, NCH * 128], F32, kind="ExternalOutput"),
    )
    with tile.TileContext(nc) as tc:
        _build_body(nc, tc, tensors)
    if not _for_sim:
        lower_extended_insts(nc)
        _split_excess_waits(nc)
    return nc


def _build_body(nc, tc, t):
    from contextlib import ExitStack
    with ExitStack() as ctx:
        const = ctx.enter_context(tc.tile_pool(name="const", bufs=1))

        nc.gpsimd.load_library(mlp)

        kT = const.tile([128, S // 16, 16], F32)
        qcb = const.tile([128, NST * QW], BF16)
        cb16 = const.tile([128, W16], BF16)
        cf32 = const.tile([128, W32], F32)
        # kT first: the stage-1 front needs it before anything else
        nc.sync.dma_start(kT[:], t["kT"][:])
        nc.sync.dma_start(cb16[:], t["cb16"][:])
        nc.sync.dma_start(cf32[:], t["cf32"][:])
        for j in range(NST):
            nc.sync.dma_start(qcb[:, j * QW:(j + 1) * QW],
                              t["qcb"][:, j * QW:(j + 1) * QW])

        def qTb(lo, n):
            w, r = divmod(lo, 1024)
            assert r + n <= 1024
            return qcb[:, QW * w + r:QW * w + r + n]

        def qTl(lo, n):
            w, r = divmod(lo, 1024)
            assert r + n <= 1024
            return qcb[:, QW * w + 1024 + r:QW * w + 1024 + r + n]

        def bias1(ch):
            w, r = divmod(ch, WCH)
            o = QW * w + 2048 + C * r
            return qcb[:, o:o + C]

        def c16(nm, lo, n):
            o = B16[nm] + lo
            return cb16[:, o:o + n]

        def c32(nm, lo, n):
            o = F32O[nm] + lo
            return cf32[:, o:o + n]

        out_d = t["out"]
        outl_d = t["outl"]

        # ---- compressed keys: kcmpT[d, c] = sum_{j<32} kT[d, 16c+j] -------
        # computed in fp32, then split hi/lo into bf16 for the 3-term matmul
        half = const.tile([128, 128], F32)
        nc.vector.tensor_reduce(half[:], kT[:], axis=AX.X, op=ALU.add)
        kcmpT = const.tile([128, C], F32)
        nc.vector.tensor_add(kcmpT[:], half[:, 0:C], half[:, 1:C + 1])
        khB = const.tile([128, C], BF16)
        nc.vector.tensor_copy(khB[:], kcmpT[:])
        khF = const.tile([128, C], F32)
        nc.vector.tensor_copy(khF[:], khB[:])
        klB = const.tile([128, C], BF16)
        nc.vector.tensor_tensor(klB[:], kcmpT[:], khF[:], op=ALU.subtract)

        gidx = ctx.enter_context(tc.tile_pool(name="gidx", bufs=2))
        kgp = ctx.enter_context(tc.tile_pool(name="kgp", bufs=3))
        vgp = ctx.enter_context(tc.tile_pool(name="vgp", bufs=3))
        s1 = ctx.enter_context(tc.tile_pool(name="s1", bufs=6))
        s1b = ctx.enter_context(tc.tile_pool(name="s1b", bufs=4))
        s2 = ctx.enter_context(tc.tile_pool(name="s2", bufs=3))
        s2o = ctx.enter_context(tc.tile_pool(name="s2o", bufs=2))

        gathered = {}
        with tc.tile_pool(name="ps_lg1", bufs=2, space="PSUM") as ps_lg1, \
             tc.tile_pool(name="ps_sc", bufs=2, space="PSUM") as ps_sc, \
             tc.tile_pool(name="ps_bt", bufs=2, space="PSUM") as ps_bt:

            # ================= stage 1: scores + argmax block ==============
            score_ps = [None] * NST

            phpl = {}
            l12h = [None]

            def s1_front(ch):
                st, sub = divmod(ch, WCH)
                co = 128 * ch
                lg1 = ps_lg1.tile([128, C], F32, tag="lg1")
                nc.tensor.matmul(lg1[:], c16("identB", 0, 128),
                                 bias1(ch),
                                 start=True, stop=False)
                nc.tensor.matmul(lg1[:], qTb(co, 128), khB[:],
                                 start=False, stop=False)
                nc.tensor.matmul(lg1[:], qTb(co, 128), klB[:],
                                 start=False, stop=False)
                nc.tensor.matmul(lg1[:], qTl(co, 128), khB[:],
                                 start=False, stop=True)
                e1f = s1.tile([128, C], F32, tag="e1f")
                nc.scalar.activation(e1f[:], lg1[:], AF.Exp, scale=SCALE1)
                if ch % 2 == 0:
                    l12h[0] = s1b.tile([128, 2], F32, tag="l12", name="l12")
                l12 = l12h[0]
                nc.vector.tensor_reduce(l12[:, ch % 2:ch % 2 + 1], e1f[:],
                                        axis=AX.X, op=ALU.add)
                if ch % 2 == 1:
                    nln2 = s1b.tile([128, 2], F32, tag="nln2")
                    nc.scalar.activation(nln2[:], l12[:], AF.Ln,
                                         bias=c32("eps20", 0, 1))
                    nc.scalar.activation(nln2[:], nln2[:], AF.Copy, scale=-1.0)
                    l12h[1:] = [nln2]
                phpl[ch] = (lg1, None)

            def s1_probs(ch):
                lg1, _ = phpl[ch]
                nln2 = l12h[1]
                p1 = s1.tile([128, C], F32, tag="p1")
                nc.scalar.activation(p1[:], lg1[:], AF.Exp, scale=SCALE1,
                                     bias=nln2[:, ch % 2:ch % 2 + 1])
                ph = s1.tile([128, C], BF16, tag="ph")
                nc.vector.tensor_copy(ph[:], p1[:])
                pl = s1.tile([128, C], BF16, tag="pl")
                nc.vector.tensor_tensor(pl[:], p1[:], ph[:], op=ALU.subtract)
                phpl[ch] = (ph, pl)

            def s1_score(ch):
                st, sub = divmod(ch, WCH)
                ph, pl = phpl.pop(ch)
                if sub == 0:
                    score_ps[st] = ps_sc.tile([128, 128], F32, tag="score",
                                              name="score")
                j = sub // 2
                sel = c16("sel32e" if sub % 2 == 0 else "sel32o", 0, 32)
                nc.tensor.matmul(score_ps[st][32 * j:32 * j + 32, 0:C],
                                 sel, ph[:],
                                 start=(sub % 2 == 0), stop=False,
                                 tile_position=(0, 32 * j))
                nc.tensor.matmul(score_ps[st][32 * j:32 * j + 32, 0:C],
                                 sel, pl[:],
                                 start=False, stop=(sub % 2 == 1),
                                 tile_position=(0, 32 * j))
                if sub == WCH - 1:
                    _argmax_gather(st)

            def _argmax_gather(st):
                score = score_ps[st]
                blk = s1.tile([128, B], F32, tag="blk")
                nc.vector.tensor_copy(blk[:], score[:, 0:125:4])
                nc.vector.tensor_tensor(blk[:], blk[:], score[:, 1:126:4],
                                        op=ALU.max)
                nc.vector.tensor_tensor(blk[:], blk[:], score[:, 2:127:4],
                                        op=ALU.max)
                nc.vector.tensor_tensor(blk[:, 0:31], blk[:, 0:31],
                                        score[:, 3:127:4], op=ALU.max)
                nc.vector.tensor_tensor(blk[:, 1:32], blk[:, 1:32],
                                        score[:, 3:127:4], op=ALU.max)
                nc.vector.tensor_add(blk[:], blk[:],
                                     c32("mmid", B * st, B))
                mx = s1b.tile([128, 1], F32, tag="mx")
                nc.vector.tensor_reduce(mx[:], blk[:], axis=AX.X, op=ALU.max)
                enc = s1.tile([128, B], F32, tag="enc")
                nc.vector.tensor_scalar(enc[:], blk[:], mx[:], 1024.0,
                                        op0=ALU.is_lt, op1=ALU.mult)
                nc.vector.tensor_tensor(enc[:], enc[:], c32("iotab", 0, B),
                                        op=ALU.add)
                bsf = s1b.tile([128, 1], F32, tag="bsf")
                nc.vector.tensor_reduce(bsf[:], enc[:], axis=AX.X, op=ALU.min)
                # K block idx: i=j -> b(even token 2j), i=64+j -> b(odd)
                rqK = s1b.tile([128, 8], BF16, tag="rqK")
                nc.vector.tensor_tensor(rqK[:], bsf[:].to_broadcast([128, 8]),
                                        c16("qmE", 0, 8), op=ALU.mult)
                ktpK = ps_bt.tile([128, 8], F32, tag="ktpK")
                nc.tensor.matmul(ktpK[:], c16("selKE", 0, 128), rqK[:],
                                 start=True, stop=True)
                idxK = gidx.tile([128, 8], I16, tag="idxK")
                nc.vector.tensor_copy(idxK[:], ktpK[:])
                # V half-slab idx: i=j -> 2*(32*b(2j)+b(2j+1)), i=64+j -> +1
                rqV = s1b.tile([128, 8], BF16, tag="rqV")
                nc.vector.tensor_tensor(rqV[:], bsf[:].to_broadcast([128, 8]),
                                        c16("qmask8", 0, 8), op=ALU.mult)
                ktpV = ps_bt.tile([128, 8], F32, tag="ktpV")
                nc.tensor.matmul(ktpV[:], c16("selP2", 0, 128), rqV[:],
                                 start=True, stop=True)
                t8 = s1b.tile([128, 8], F32, tag="t8")
                nc.vector.tensor_add(t8[:], ktpV[:], c32("off1", 0, 8))
                idxV = gidx.tile([128, 8], I16, tag="idxV")
                nc.vector.tensor_copy(idxV[:], t8[:])
                kg = kgp.tile([128, 64, 128], BF16, tag="kg")
                nc.gpsimd.dma_gather(kg[:], t["k16"][:], idxK[:],
                                     128, 128, 64 * 128, transpose=True)
                vg = vgp.tile([128, 64, 128], BF16, tag="vg")
                nc.gpsimd.dma_gather(vg[:], t["vptab"][:], idxV[:],
                                     128, 128, 64 * 128, transpose=True)
                gathered[st] = (kg, vg)

            LAG = 4
            for ch in range(NCH + LAG):
                if ch < NCH:
                    s1_front(ch)
                    if ch % 2 == 1:
                        s1_probs(ch - 1)
                        s1_probs(ch)
                if ch >= LAG:
                    s1_score(ch - LAG)

        # ================= stage 2: block-sparse attention =================
        with tc.tile_pool(name="psA", bufs=2, space="PSUM") as psA, \
             tc.tile_pool(name="psB", bufs=2, space="PSUM") as psB, \
             tc.tile_pool(name="psO", bufs=2, space="PSUM") as psO, \
             tc.tile_pool(name="psL", bufs=2, space="PSUM") as psL:

            s2st = {}

            def s2_front(g):
                kg, vg = gathered[g // 2]
                jo = 32 * (g % 2)          # wave-pair offset of this group
                co = GC * g
                # fixed 128 keys (block0 | block qb-1), transposed logits
                lgA = psA.tile([128, GC], F32, tag="lgA")
                nc.tensor.matmul(lgA[:], c16("kabT", 128 * g, 128),
                                 qTb(co, GC), start=True, stop=True)
                # dynamic pair logits: 2 tokens x 64 keys per matmul
                lgB = psB.tile([128, GC], F32, tag="lgB")
                for j in range(32):
                    jj = jo + j
                    nc.tensor.matmul(
                        lgB[:, 16 * j:16 * j + 16],
                        kg[:, :, jj:jj + 65:64],
                        qTb(co + 16 * j, 16),
                        start=True, stop=True)
                eA = s2.tile([128, GC], BF16, tag="eA")
                nc.scalar.activation(eA[:], lgA[:], AF.Exp, scale=SCALE,
                                     bias=c32("b2a", g, 1))
                # kill cross-token quadrants, then exp
                nc.vector.tensor_add(lgB[:], lgB[:], c32("maskc", 0, GC))
                ptB = s2.tile([128, GC], BF16, tag="ptB")
                nc.scalar.activation(ptB[:], lgB[:], AF.Exp, scale=SCALE,
                                     bias=c32("b2b", g, 1))
                s2st[g] = (eA, ptB)

            def s2_tail(g):
                kg, vg = gathered[g // 2]
                jo = 32 * (g % 2)
                co = GC * g
                eA, ptB = s2st.pop(g)
                # softmax denominators (column sums over both key groups)
                lr = psL.tile([1, GC], F32, tag="lr")
                nc.tensor.matmul(lr[:], c16("onesb", 0, 1), eA[:],
                                 start=True, stop=False)
                nc.tensor.matmul(lr[:], c16("onesb", 0, 1), ptB[:],
                                 start=False, stop=True)
                louts = s2o.tile([1, GC], F32, tag="louts")
                nc.vector.tensor_copy(louts[:], lr[:])
                nc.sync.dma_start(outl_d[:, co:co + GC], louts[:])
                # PV accumulation (transposed output [d, row])
                oT = psO.tile([128, GC], F32, tag="oT")
                nc.tensor.matmul(oT[:], c16("vab", 128 * g, 128), eA[:],
                                 start=True, stop=False)
                for j in range(32):
                    jj = jo + j
                    nc.tensor.matmul(
                        oT[:, 16 * j:16 * j + 16],
                        vg[:, :, jj:jj + 65:64],
                        ptB[:, 16 * j:16 * j + 16],
                        start=False, stop=(j == 31))
                outc = s2o.tile([128, GC], BF16, tag="outc")
                nc.vector.tensor_copy(outc[:], oT[:])
                nc.sync.dma_start(out_d[:, co:co + GC], outc[:])

            for g in range(NGRP + 1):
                if g < NGRP:
                    s2_front(g)
                if g >= 1:
                    s2_tail(g - 1)


_NC_CACHE = None


def _get_program():
    global _NC_CACHE
    if _NC_CACHE is None:
        _NC_CACHE = build_program()
    return _NC_CACHE


def _make_tables(k, v, h):
    """Per-KV-head gather tables (16KB elems):
    k16   [B, 64*128]      K block natural [key, d] -> transpose-gather [d,key]
    vptab [2*B*B, 64*128]  V-pair half-slabs, row 2p+h (p = 32*be+bo):
                           elem[e, pp] = V_{b(pp%2)}[pp//2, d=2e+h]
                           (pp = interleaved pair keys 2k+t, d split by parity)
    """
    kh = k[:, h, :].astype(ml_dtypes.bfloat16)
    vh = v[:, h, :].astype(ml_dtypes.bfloat16)
    k16 = np.ascontiguousarray(kh.reshape(B, BLOCK * D))
    Vt2 = vh.reshape(B, BLOCK, D).transpose(0, 2, 1)       # [b, d, key]
    A = np.broadcast_to(Vt2[:, None, :, :], (B, B, D, BLOCK))
    Bb = np.broadcast_to(Vt2[None, :, :, :], (B, B, D, BLOCK))
    WT = np.stack([A, Bb], axis=-1).reshape(B * B, D, 128)  # [p, d, pp]
    vptab = np.ascontiguousarray(
        WT.reshape(B * B, 64, 2, 128).transpose(0, 2, 1, 3)
    ).reshape(2 * B * B, 64 * 128)
    return k16, vptab


def _make_core_inputs(q, k, v, h, part, k16, vptab):
    qbs = [part + NPART * j for j in range(NQ // BLOCK)]
    ls = np.concatenate([np.arange(BLOCK * b, BLOCK * b + BLOCK) for b in qbs])
    qc = q[ls][:, h * G:(h + 1) * G, :].reshape(NQ * G, D)
    qT = np.ascontiguousarray(qc.T)
    qTb = qT.astype(ml_dtypes.bfloat16)
    qTl = (qT - qTb.astype(np.float32)).astype(ml_dtypes.bfloat16)
    kh = k[:, h, :]
    kT = np.ascontiguousarray(kh.T)
    vh = v[:, h, :]

    qb_g = np.array(qbs)                       # query block per group
    qbf = np.maximum(qb_g - 1, 0)

    kabT = np.empty((128, NGRP * 128), np.float32)
    vab = np.empty((128, NGRP * 128), np.float32)
    for g in range(NGRP):
        kabT[:, 128 * g:128 * g + 64] = kT[:, 0:64]
        kabT[:, 128 * g + 64:128 * g + 128] = \
            kT[:, 64 * qbf[g]:64 * qbf[g] + 64]
        vab[0:64, 128 * g:128 * g + 128] = vh[0:64]
        vab[64:128, 128 * g:128 * g + 128] = \
            vh[64 * qbf[g]:64 * qbf[g] + 64]
    kabT = kabT.astype(ml_dtypes.bfloat16)
    vab = vab.astype(ml_dtypes.bfloat16)

    b2a = np.empty((128, NGRP), np.float32)
    b2a[0:64] = np.where(qb_g >= 1, 0.0, NEG)[None, :]
    b2a[64:128] = np.where(qb_g >= 2, 0.0, NEG)[None, :]
    b2b = np.broadcast_to(
        np.where(qb_g >= 3, 0.0, NEG).astype(np.float32), (128, NGRP)).copy()

    # stage-1 visibility bias: compressed key c visible iff 16c+31 <= s
    rows_s = ls[(QPC * np.arange(NCH)[None, :] + np.arange(128)[:, None] // G)]
    thr = np.floor((rows_s.astype(np.float64) - (KERNEL - 1)) / STRIDE)
    vis = np.arange(C)[None, :, None] <= thr.T[:, None, :]  # [NCH, C, 128]
    bias1 = np.where(vis, 0.0, NEG)
    bias1 = np.ascontiguousarray(
        bias1.transpose(2, 0, 1).reshape(128, NCH * C)).astype(
            ml_dtypes.bfloat16)
    identB = np.eye(128, dtype=ml_dtypes.bfloat16)

    qb_of_li = ls // BLOCK
    mmid = np.full((128, NST * B), -1e38, np.float32)
    for sti in range(NST):
        qb_rows = qb_of_li[128 * sti + np.arange(128)]
        allowed = (np.arange(B)[None, :] >= 1) & \
                  (np.arange(B)[None, :] <= qb_rows[:, None] - 2)
        allowed[~allowed.any(axis=1), 1] = True
        mmid[:, B * sti:B * sti + B] = np.where(allowed, 0.0, -1e38)

    iotab = np.broadcast_to(np.arange(B, dtype=np.float32), (128, B)).copy()
    r128 = np.arange(128)
    sel32e = (r128[:, None] // 8 == np.arange(32)[None, :]
              ).astype(ml_dtypes.bfloat16)
    sel32o = (r128[:, None] // 8 == np.arange(32)[None, :] - 16
              ).astype(ml_dtypes.bfloat16)
    selKE = ((r128[:, None] % 32) // 2 == r128[None, :] % 16
             ).astype(ml_dtypes.bfloat16)
    qmE = ((r128[:, None] // 32 == np.arange(8)[None, :] % 4) &
           (r128[:, None] % 2 == np.arange(8)[None, :] // 4)
           ).astype(ml_dtypes.bfloat16)
    selP2 = (64.0 * (r128[:, None] % 32 == 2 * (r128[None, :] % 16)) +
             2.0 * (r128[:, None] % 32 == 2 * (r128[None, :] % 16) + 1)
             ).astype(ml_dtypes.bfloat16)
    qmask8 = (r128[:, None] // 32 == np.arange(8)[None, :] % 4
              ).astype(ml_dtypes.bfloat16)
    off1 = np.broadcast_to(
        1.0 * (np.arange(8) >= 4).astype(np.float32), (128, 8)).copy()
    cc = np.arange(GC)
    maskc = np.where(r128[:, None] % 2 == (cc[None, :] % 16) // 8,
                     0.0, NEG).astype(np.float32)
    onesb = np.ones((128, 1), ml_dtypes.bfloat16)

    qcb = np.empty((128, NST * QW), ml_dtypes.bfloat16)
    for w in range(NST):
        qcb[:, QW * w:QW * w + 1024] = qTb[:, 1024 * w:1024 * (w + 1)]
        qcb[:, QW * w + 1024:QW * w + 2048] = qTl[:, 1024 * w:1024 * (w + 1)]
        qcb[:, QW * w + 2048:QW * (w + 1)] = \
            bias1[:, C * WCH * w:C * WCH * (w + 1)]
    cb16 = np.zeros((128, W16), ml_dtypes.bfloat16)
    for nm, arr in [("kabT", kabT), ("vab", vab), ("sel32e", sel32e),
                    ("sel32o", sel32o), ("selKE", selKE), ("qmE", qmE),
                    ("selP2", selP2), ("qmask8", qmask8),
                    ("onesb", np.ones((128, 1), ml_dtypes.bfloat16)),
                    ("identB", identB)]:
        cb16[:, B16[nm]:B16[nm] + arr.shape[1]] = arr
    cf32 = np.zeros((128, W32), np.float32)
    for nm, arr in [("b2a", b2a), ("b2b", b2b), ("mmid", mmid),
                    ("iotab", iotab), ("off1", off1), ("maskc", maskc),
                    ("eps20", np.full((128, 1), 1e-20, np.float32))]:
        cf32[:, F32O[nm]:F32O[nm] + arr.shape[1]] = arr
    return {"kT": kT, "qcb": qcb, "cb16": cb16, "cf32": cf32,
            "k16": k16, "vptab": vptab}, ls


def kernel(q, k, v, _profile=False):
    q = np.asarray(q, dtype=np.float32)
    k = np.asarray(k, dtype=np.float32)
    v = np.asarray(v, dtype=np.float32)
    nc = _get_program()

    tabs = [_make_tables(k, v, h) for h in range(HKV)]

    in_maps = []
    ls_per_core = []
    for c in range(NCORES):
        h, part = divmod(c, NPART)
        im, ls = _make_core_inputs(q, k, v, h, part, *tabs[h])
        in_maps.append(im)
        ls_per_core.append(ls)

    kw = dict(trace=True) if _profile else {}
    res = run_bass_kernel_spmd(nc, in_maps, list(range(NCORES)), **kw)

    out = np.zeros((S, HQ, D), dtype=np.float32)
    for c in range(NCORES):
        h, part = divmod(c, NPART)
        oc = np.asarray(res.results[c]["out"], dtype=np.float32)  # [128, 4096]
        l = np.asarray(res.results[c]["outl"], dtype=np.float32)  # [1, 4096]
        oc = oc / np.maximum(l, 1e-30)
        ocr = oc.reshape(D, NCH, QPC, G).transpose(1, 2, 3, 0)  # [NCH,16,G,D]
        out[ls_per_core[c], h * G:(h + 1) * G, :] = ocr.reshape(NQ, G, D)
    if _profile:
        return out, res
    return out
